# revision 59
# baseline (speedup 1.0000x reference)
"""Trainium2 Bass kernel for nn_Attention_89137751261457.

Full attention with 3D RoPE + QK RMSNorm, B=1, N=4096, C=2048, 16 heads,
head_dim=128. Sharded tensor-parallel by head across 8 NeuronCores
(2 heads per core); the output projection is computed per-core on the
head slice and the 8 partial outputs are summed on the host.

v3 design (f16 everywhere, engine-balanced):
  - all matmuls f16 (same PE cost as bf16, much lower error)
  - V computed via transposed matmuls (tokens on partitions) - no
    separate PE transposes or extra copies
  - softmax denominator via a DVE f16 pairwise add tree (frees ~106us
    of PE ones-matmul time), one (1,512) ones-matmul per q-block
  - elementwise work split: ACT (exp, psum copies), DVE (psum mults,
    den tree), Pool/gpsimd (SBUF-only mults + broadcasts)
  - phase1(head1) interleaved with attention(head0)

Self-contained: hardcodes all shapes; imports only numpy/ml_dtypes/concourse.
"""

import numpy as np
import ml_dtypes

import concourse.bass as bass  # noqa: F401
import concourse.bacc as bacc
import concourse.mybir as mybir
import concourse.tile as tile  # noqa: F401
from concourse.bass_utils import run_bass_kernel_spmd
from concourse.tile import TileContext

F16 = np.float16
BF16 = ml_dtypes.bfloat16

NUM_HEADS = 16
DIM = 2048
N = 4096
HD = 128          # head dim
P = 128           # partitions
NCORES = 8
HPC = 2           # heads per core
RMS_EPS = 1e-6
ROPE_THETA = 10000.0
SCALE = float(HD) ** -0.5

KO = DIM // P     # 16 contraction chunks
NB = N // 512     # 8 n-chunks of 512 (phase 1)
QB = N // 512     # 8 q-blocks of 512 (phase 2)
MI = N // P       # 32 m-chunks of 128
COLS = 3 * HPC    # 6 col chunks of 128 (q0,k0,v0,q1,k1,v1)
EXP_BIAS = -2.0   # softmax exp bias (cancels in numerator/denominator)

_CACHE = {}


# --------------------------------------------------------------------------
# host-side helpers
# --------------------------------------------------------------------------

def _rope_ext_tables(T, H, W):
    """cos_ext, sin_ext of shape (HD, N): extended interleaved RoPE tables.

    q_rot[d, n] = cos_ext[d, n] * q[d, n] + sin_ext[d, n] * q[pair(d), n]
    where pair(2i) = 2i+1, pair(2i+1) = 2i.
    """
    dt_ = HD // 2
    dh = HD // 4
    dw = HD - dt_ - dh

    def ax(L, d):
        inv = 1.0 / (ROPE_THETA ** (np.arange(0, d, 2, dtype=np.float32) / d))
        return np.arange(L, dtype=np.float32)[:, None] * inv

    ft = np.broadcast_to(ax(T, dt_)[:, None, None, :], (T, H, W, dt_ // 2))
    fh = np.broadcast_to(ax(H, dh)[None, :, None, :], (T, H, W, dh // 2))
    fw = np.broadcast_to(ax(W, dw)[None, None, :, :], (T, H, W, dw // 2))
    f = np.concatenate([ft, fh, fw], axis=-1).reshape(T * H * W, HD // 2)
    cos = np.cos(f).astype(np.float32).T   # (64, N)
    sin = np.sin(f).astype(np.float32).T   # (64, N)
    n_tot = T * H * W
    cos_ext = np.repeat(cos, 2, axis=0)    # (128, N)
    sin_ext = np.empty((HD, n_tot), np.float32)
    sin_ext[0::2] = -sin
    sin_ext[1::2] = sin
    return cos_ext, sin_ext


def _pair_swap(v):
    """Swap adjacent pairs of a (128,) vector."""
    return v.reshape(HD // 2, 2)[:, ::-1].reshape(HD)


# --------------------------------------------------------------------------
# device program
# --------------------------------------------------------------------------

def _build_nc(with_bias=False):
    nc = bacc.Bacc("TRN2", target_bir_lowering=False, debug=False)
    f32 = mybir.dt.float32
    f16 = mybir.dt.float16

    Exp = mybir.ActivationFunctionType.Exp
    Log = mybir.ActivationFunctionType.Ln
    Copy = mybir.ActivationFunctionType.Copy
    MULT = mybir.AluOpType.mult
    ADD = mybir.AluOpType.add

    xt = nc.dram_tensor("xt", [DIM, N], f16, kind="ExternalInput")
    wt = nc.dram_tensor("wt", [P, COLS, KO, P], f16, kind="ExternalInput")
    bqkv = nc.dram_tensor("bqkv", [P, COLS], f32, kind="ExternalInput")
    wp = nc.dram_tensor("wp", [HPC, P, DIM], f16, kind="ExternalInput")
    cosq = nc.dram_tensor("cosq", [P, N], f16, kind="ExternalInput")
    sinq = nc.dram_tensor("sinq", [P, N], f16, kind="ExternalInput")
    cosk = nc.dram_tensor("cosk", [P, N], f16, kind="ExternalInput")
    sink = nc.dram_tensor("sink", [P, N], f16, kind="ExternalInput")
    ones16 = nc.dram_tensor("ones16", [P, 1], f16, kind="ExternalInput")
    perm = nc.dram_tensor("perm", [P, P], f16, kind="ExternalInput")
    epsc = nc.dram_tensor("epsc", [1, 1], f32, kind="ExternalInput")
    ebias = nc.dram_tensor("ebias", [P, 1], f32, kind="ExternalInput")
    onesr = nc.dram_tensor("onesr", [1, P], f16, kind="ExternalInput")
    bqvr = nc.dram_tensor("bqvr", [1, COLS * P], f16, kind="ExternalInput")
    out_p = nc.dram_tensor("out_p", [N, DIM], f16, kind="ExternalOutput")

    xt_v = xt[:].rearrange("(ko p) n -> p ko n", p=P)    # (128, 16, 4096)

    from concourse.hw_specs import get_activation_tables
    tabs = get_activation_tables(nc.m.arch)
    need = {mybir.ActivationFunctionType.Exp, mybir.ActivationFunctionType.Ln,
            mybir.ActivationFunctionType.Copy}
    set_id = next((i for i, (nm, s) in enumerate(tabs.items())
                   if need.issubset(s)), None)

    with TileContext(nc) as tc:
      if set_id is not None:
          nc.scalar.add_instruction(mybir.InstLoadActFuncSet(
              name=nc.get_next_instruction_name(), act_func_set_id=set_id,
              ins=[], outs=[]))
      with tc.tile_pool(name="persist", bufs=1) as pers:
        wt_sb = pers.tile([P, COLS, KO, P], f16, tag="wt_sb")
        # column order: k0, v0, q0 first (phase-1 head-0 kv pass needs them)
        for c6 in (1, 2):
            nc.sync.dma_start(wt_sb[:, c6], wt[:, c6])
        wp_sb = [pers.tile([P, DIM], f16, tag=f"wp{h}", name=f"wp_sb{h}")
                 for h in range(HPC)]
        ones_sb = pers.tile([P, 1], f16, tag="ones_sb")
        nc.sync.dma_start(ones_sb[:], ones16[:])
        perm_sb = pers.tile([P, P], f16, tag="perm_sb")
        nc.sync.dma_start(perm_sb[:], perm[:])
        eps_sb = pers.tile([1, 1], f32, tag="eps_sb")
        nc.sync.dma_start(eps_sb[:], epsc[:])
        eb_sb = pers.tile([P, 1], f32, tag="eb_sb")
        nc.sync.dma_start(eb_sb[:], ebias[:])
        bq_sb = pers.tile([P, COLS], f32, tag="bq_sb")
        nc.sync.dma_start(bq_sb[:], bqkv[:])
        onesr_sb = pers.tile([1, P], f16, tag="onesr_sb")
        nc.sync.dma_start(onesr_sb[:], onesr[:])
        bqvr_sb = pers.tile([1, COLS * P], f16, tag="bqvr_sb")
        nc.sync.dma_start(bqvr_sb[:], bqvr[:])

        qT = [pers.tile([P, N], f16, tag=f"qT{h}", name=f"qT{h}")
              for h in range(HPC)]
        kT = [pers.tile([P, N], f16, tag=f"kT{h}", name=f"kT{h}")
              for h in range(HPC)]
        vn = [pers.tile([P, MI, HD], f16, tag=f"vn{h}", name=f"vn{h}")
              for h in range(HPC)]
        oT = [pers.tile([P, N], f16, tag=f"oT{h}", name=f"oT{h}")
              for h in range(HPC)]

        cs_dram = {("q", 0): cosq, ("q", 1): sinq,
                   ("k", 0): cosk, ("k", 1): sink}

        def phase1(hh, nb, p1, wk, ps_qkv, ps_qs, ps_sm, which="all"):
            """qkv + rmsnorm + rope (+ transposed v) for head hh, block nb.

            which: "all", "kv" (k column + v), or "q" (q column only).
            """
            nsl = slice(nb * 512, (nb + 1) * 512)
            xt_t = p1.tile([P, KO, 512], f16, tag="xt_t",
                           name=f"xt_{hh}_{nb}_{which}")
            nsp = 8 if (nb == 0 and which == "kv") else 2
            for kh in range(nsp):
                kw = KO // nsp
                nc.sync.dma_start(xt_t[:, kh * kw:(kh + 1) * kw, :],
                                  xt_v[:, kh * kw:(kh + 1) * kw, nsl])
            cs_t = {}
            tags = [] if which == "kv" else ["q"]
            if which != "q":
                tags.append("k")
            for tg in tags:
                for ci in range(2):
                    t = p1.tile([P, 512], f16, tag=f"cs_{tg}_{ci}",
                                name=f"cs_{tg}_{ci}")
                    nc.sync.dma_start(t[:], cs_dram[(tg, ci)][:, nsl])
                    cs_t[(tg, ci)] = t

            pqbs = {}
            for tag in tags:
                t3 = 0 if tag == "q" else 1
                col = hh * 3 + t3
                pq = ps_qkv.tile([P, 512], f32, tag="pq", name="pq")
                for k in range(KO):
                    nc.tensor.matmul(
                        pq[:],
                        lhsT=wt_sb[:, col, k, :],
                        rhs=xt_t[:, k],
                        start=(k == 0), stop=(k == KO - 1))
                # pqb: psum -> f16 sbuf (DVE); optional bias add
                pqb = wk.tile([P, 512], f16, tag="pqb", bufs=4)
                if with_bias:
                    nc.vector.tensor_scalar_add(pqb[:], pq[:],
                                                bq_sb[:, col:col + 1])
                else:
                    nc.vector.tensor_copy(pqb[:], pq[:])
                pqbs[tag] = pqb

            if which != "q":
                # v via transposed matmuls (fills PE while rms chains run)
                vcol = hh * 3 + 2
                pv = ps_qkv.tile([P, 4, HD], f32, tag="pq", name="pv")
                for c4 in range(4):
                    for k in range(KO):
                        nc.tensor.matmul(
                            pv[:, c4, :],
                            lhsT=xt_t[:, k, c4 * P:(c4 + 1) * P],
                            rhs=wt_sb[:, vcol, k, :],
                            start=(k == 0),
                            stop=(k == KO - 1 and not with_bias))
                    if with_bias:
                        nc.tensor.matmul(
                            pv[:, c4, :], lhsT=onesr_sb[0:1, :],
                            rhs=bqvr_sb[0:1, vcol * P:(vcol + 1) * P],
                            start=False, stop=True)
                dstv = vn[hh][:, nb * 4:(nb + 1) * 4, :]
                if hh == 0:
                    nc.scalar.activation(
                        dstv.rearrange("p a b -> p (a b)"),
                        pv[:].rearrange("p a b -> p (a b)"), Copy)
                else:
                    nc.vector.tensor_copy(
                        dstv.rearrange("p a b -> p (a b)"),
                        pv[:].rearrange("p a b -> p (a b)"))

            for tag in tags:
                t3 = 0 if tag == "q" else 1
                pqb = pqbs[tag]
                # rms: rs = (mean(pqb^2) + eps) ** -0.5 on (1, 512)
                q2b = wk.tile([P, 512], f16, tag="q2b", bufs=2)
                nc.gpsimd.tensor_tensor(q2b[:], pqb[:], pqb[:], MULT)
                pssq = ps_sm.tile([1, 512], f32, tag="qsm", name="pssq")
                nc.tensor.matmul(pssq[:], lhsT=ones_sb[:],
                                 rhs=q2b[:], start=True, stop=True)
                lms = wk.tile([1, 512], f32, tag="lms", bufs=2)
                nc.scalar.activation(lms[:], pssq[:], Log,
                                     bias=eps_sb[:], scale=1.0 / HD)
                rs = wk.tile([1, 512], f16, tag="rs", bufs=2)
                nc.scalar.activation(rs[:], lms[:], Exp, scale=-0.5)
                rsb = wk.tile([P, 512], f16, tag="rsb", bufs=3)
                nc.gpsimd.partition_broadcast(rsb[:], rs[:])
                # rope: dst = (pqb*cos + swap(pqb)*sin) * rsb
                pqs = ps_qs.tile([P, 512], f32, tag="qsm", name="pqs")
                nc.tensor.matmul(pqs[:], lhsT=perm_sb[:],
                                 rhs=pqb[:], start=True, stop=True)
                t1 = wk.tile([P, 512], f16, tag="t1", bufs=3)
                nc.gpsimd.tensor_tensor(t1[:], pqb[:], cs_t[(tag, 0)][:],
                                        MULT)
                t2 = wk.tile([P, 512], f16, tag="t2", bufs=3)
                nc.vector.tensor_tensor(t2[:], pqs[:], cs_t[(tag, 1)][:],
                                        MULT)
                t3t = wk.tile([P, 512], f16, tag="t3", bufs=2)
                nc.vector.tensor_tensor(t3t[:], t1[:], t2[:], ADD)
                dst = qT[hh] if t3 == 0 else kT[hh]
                nc.vector.tensor_tensor(dst[:, nsl], t3t[:], rsb[:], MULT)

            return

        def attention(h, qb, p2, p2s, ps_s, ps_o, ps_den,
                      pe_den=False):
            qsl = slice(qb * 512, (qb + 1) * 512)
            po = ps_o.tile([P, 512], f32, tag="po")
            pden_pe = (ps_den.tile([1, 512], f32, tag="qsm", name="pden_pe")
                       if pe_den else None)
            accs = [None, None, None, None]
            for mp in range(MI // 2):
                m0 = 2 * mp
                ps = ps_s.tile([P, 2, 512], f32, tag="ps")
                for u in range(2):
                    nc.tensor.matmul(
                        ps[:, u, :],
                        lhsT=kT[h][:, (m0 + u) * P:(m0 + u + 1) * P],
                        rhs=qT[h][:, qsl], start=True, stop=True)
                at = p2.tile([P, 2, 512], f16, tag="at", bufs=8)
                nc.scalar.activation(
                    at[:].rearrange("p a b -> p (a b)"),
                    ps[:].rearrange("p a b -> p (a b)"),
                    Exp, scale=SCALE, bias=eb_sb[:])
                for u in range(2):
                    nc.tensor.matmul(
                        po[:], lhsT=vn[h][:, m0 + u, :], rhs=at[:, u, :],
                        start=(m0 + u == 0), stop=(m0 + u == MI - 1))
                # denominator tree on DVE: pair-sum then 4 rotating accs
                tadd = p2.tile([P, 512], f16, tag="tadd", bufs=6)
                nc.vector.tensor_tensor(tadd[:], at[:, 0, :], at[:, 1, :],
                                        ADD)
                g = mp % 4
                if accs[g] is None:
                    accs[g] = tadd
                else:
                    nacc = p2.tile([P, 512], f16, tag=f"acc{g}", bufs=2)
                    nc.vector.tensor_tensor(nacc[:], accs[g][:], tadd[:],
                                            ADD)
                    accs[g] = nacc
            if pe_den:
                pden = pden_pe
            else:
                s01 = p2.tile([P, 512], f16, tag="s01", bufs=2)
                nc.vector.tensor_tensor(s01[:], accs[0][:], accs[1][:], ADD)
                s23 = p2.tile([P, 512], f16, tag="s23", bufs=2)
                nc.vector.tensor_tensor(s23[:], accs[2][:], accs[3][:], ADD)
                sall = p2.tile([P, 512], f16, tag="sall", bufs=2)
                nc.vector.tensor_tensor(sall[:], s01[:], s23[:], ADD)
                pden = ps_den.tile([1, 512], f32, tag="qsm", name="pden")
                _p = tc.cur_priority
                tc.cur_priority = _p + 250
                nc.tensor.matmul(pden[:], lhsT=ones_sb[:], rhs=sall[:],
                                 start=True, stop=True)
                tc.cur_priority = _p
            po_sb = p2s.tile([P, 512], f16, tag="po_sb", bufs=2)
            with tc.high_priority(offset=60):
                if h == 0:
                    nc.scalar.activation(po_sb[:], po[:], Copy)
                else:
                    nc.vector.tensor_copy(po_sb[:], po[:])
            rden = p2s.tile([1, 512], f32, tag="rden")
            nc.vector.reciprocal_approx_fast(rden[:], pden[:])
            rdb = p2s.tile([P, 512], f32, tag="rdb")
            nc.gpsimd.partition_broadcast(rdb[:], rden[:])
            nc.vector.tensor_tensor(oT[h][:, qsl], po_sb[:], rdb[:], MULT)

        def proj(qb, p3, ps_out):
            for qc in range(qb * 4, qb * 4 + 4):
                osb = p3.tile([P, DIM], f16, tag="osb")
                for ob in range(4):
                    obsl = slice(ob * 512, (ob + 1) * 512)
                    pout = ps_out.tile([P, 512], f32, tag="pout", bufs=2)
                    for h in range(HPC):
                        nc.tensor.matmul(
                            pout[:],
                            lhsT=oT[h][:, qc * P:(qc + 1) * P],
                            rhs=wp_sb[h][:, obsl],
                            start=(h == 0), stop=(h == HPC - 1))
                    if ob % 2 == 0:
                        nc.scalar.activation(osb[:, obsl], pout[:], Copy)
                    else:
                        nc.vector.tensor_copy(osb[:, obsl], pout[:])
                nc.sync.dma_start(out_p[qc * P:(qc + 1) * P, :], osb[:])

        for h in range(HPC):
            nc.sync.dma_start(wp_sb[h][:], wp[h])
        with tc.tile_pool(name="p2_sb", bufs=2) as p2, \
             tc.tile_pool(name="p2_sm", bufs=2) as p2s, \
             tc.tile_pool(name="ps_s", bufs=2, space="PSUM") as ps_s, \
             tc.tile_pool(name="ps_o", bufs=1, space="PSUM") as ps_o, \
             tc.tile_pool(name="ps_den", bufs=1, space="PSUM") as ps_den:
            ps_qs = ps_den
            ps_sm = ps_den
            with tc.tile_pool(name="p1_sb", bufs=2) as p1, \
                 tc.tile_pool(name="p1_wk", bufs=2) as wk, \
                 tc.tile_pool(name="ps_qkv", bufs=2, space="PSUM") as ps_qkv:
                phase1(0, 0, p1, wk, ps_qkv, ps_qs, ps_sm, which="kv")
                for c6 in (0, 4, 5, 3):
                    nc.sync.dma_start(wt_sb[:, c6], wt[:, c6])
                for nb in range(1, NB):
                    phase1(0, nb, p1, wk, ps_qkv, ps_qs, ps_sm, which="kv")
                phase1(0, 0, p1, wk, ps_qkv, ps_qs, ps_sm, which="q")
                for qb in range(QB):
                    if qb + 1 < QB:
                        phase1(0, qb + 1, p1, wk, ps_qkv, ps_qs, ps_sm,
                               which="q")
                    phase1(1, qb, p1, wk, ps_qkv, ps_qs, ps_sm)
                    attention(0, qb, p2, p2s, ps_s, ps_o, ps_den)
            with tc.tile_pool(name="p3_sb", bufs=3) as p3, \
                 tc.tile_pool(name="ps_out", bufs=1, space="PSUM") as ps_out:
                for qb in range(QB):
                    attention(1, qb, p2, p2s, ps_s, ps_o, ps_den)
                    proj(qb, p3, ps_out)

    nc.compile()
    return nc


# --------------------------------------------------------------------------
# host wrapper
# --------------------------------------------------------------------------

def _prep_in_maps(x, qkv_w, qkv_b, q_norm_w, k_norm_w, proj_w, T, H, W):
    x2 = np.ascontiguousarray(x[0].T).astype(F16)           # (2048, 4096)
    cos_ext, sin_ext = _rope_ext_tables(T, H, W)
    wq = q_norm_w.astype(np.float32)
    wk_ = k_norm_w.astype(np.float32)
    cosq = (cos_ext * wq[:, None]).astype(F16)
    sinq = (sin_ext * _pair_swap(wq)[:, None]).astype(F16)
    cosk = (cos_ext * wk_[:, None]).astype(F16)
    sink = (sin_ext * _pair_swap(wk_)[:, None]).astype(F16)

    ones_16 = np.ones((P, 1), F16)
    permm = np.zeros((P, P), np.float32)
    idx = np.arange(P)
    pair = idx ^ 1
    permm[pair, idx] = 1.0  # psum_qs[m,n] = sum_k perm[k,m] q[k,n] = q[pair(m),n]
    permm = permm.astype(F16)

    in_maps = []
    for c in range(NCORES):
        h0 = HPC * c
        blocks = []
        bias_blocks = []
        for h in (h0, h0 + 1):
            for t3 in range(3):
                r0 = t3 * DIM + h * HD
                blocks.append(qkv_w[r0:r0 + HD])
                bias_blocks.append(qkv_b[r0:r0 + HD])
        wt_c = np.stack(blocks, axis=0)                     # (6, 128, 2048)
        # (P, COLS, KO, P): [p, c6, ko, j] = W[c6][j, ko*128+p]
        wt_c = np.ascontiguousarray(
            wt_c.reshape(COLS, P, KO, P).transpose(3, 0, 2, 1)).astype(F16)
        bq_c = np.stack(bias_blocks, axis=1).astype(np.float32)  # (128, 6)
        wp_c = np.stack(
            [np.ascontiguousarray(proj_w[:, (h0 + h) * HD:(h0 + h + 1) * HD].T)
             for h in range(HPC)], axis=0).astype(F16)      # (2, 128, 2048)
        in_maps.append({
            "xt": x2, "wt": wt_c, "bqkv": bq_c, "wp": wp_c,
            "cosq": cosq, "sinq": sinq, "cosk": cosk, "sink": sink,
            "ones16": ones_16, "perm": permm,
            "epsc": np.full((1, 1), RMS_EPS, np.float32),
            "onesr": np.ones((1, P), F16),
            "bqvr": np.concatenate(bias_blocks)[None, :].astype(F16),
            "ebias": np.full((P, 1), EXP_BIAS, np.float32),
        })
    return in_maps


def kernel(x, qkv_w, qkv_b, q_norm_w, k_norm_w, proj_w, proj_b, T, H, W):
    x = np.asarray(x)
    T, H, W = int(T), int(H), int(W)
    assert x.shape == (1, N, DIM) and T * H * W == N

    qkv_b = np.asarray(qkv_b)
    with_bias = bool(np.any(qkv_b))
    key = ("nc", with_bias)
    if key not in _CACHE:
        _CACHE[key] = _build_nc(with_bias=with_bias)
        _CACHE["nc"] = _CACHE[key]
    nc = _CACHE[key]

    in_maps = _prep_in_maps(
        x, np.asarray(qkv_w), qkv_b, np.asarray(q_norm_w),
        np.asarray(k_norm_w), np.asarray(proj_w), T, H, W)

    res = run_bass_kernel_spmd(nc, in_maps, core_ids=list(range(NCORES)))
    out = np.zeros((N, DIM), np.float64)
    for c in range(NCORES):
        out += res.results[c]["out_p"].astype(np.float64)
    out = out.astype(np.float32) + np.asarray(proj_b, np.float32)[None, :]
    return out[None].astype(x.dtype)


# revision 74
# speedup vs baseline: 1.0226x; 1.0226x over previous
"""Trainium2 Bass kernel for nn_Attention_89137751261457.

Full attention with 3D RoPE + QK RMSNorm, B=1, N=4096, C=2048, 16 heads,
head_dim=128. Sharded tensor-parallel by head across 8 NeuronCores
(2 heads per core); the output projection is computed per-core on the
head slice and the 8 partial outputs are summed on the host.

Design (f16 everywhere, engine-balanced, ~512us modeled / 8 cores):
  - all matmuls f16 (same PE cost as bf16, ~10x lower error than bf16)
  - V computed via transposed matmuls (tokens on partitions) - no
    separate PE transposes or extra copies
  - softmax denominator via a DVE f16 pairwise add tree (frees ~106us
    of PE ones-matmul time), one (1,512) ones-matmul per q-block
  - elementwise work split: ACT (exp, psum copies), DVE (psum mults,
    den tree), Pool/gpsimd (SBUF-only mults + broadcasts)
  - single preloaded combined ln+exp activation table (avoids 64
    table reloads at 1283ns each)
  - schedule: head-0 K/V columns first, Q columns pipelined one block
    ahead of head-0 attention, head-1 phase 1 interleaved with head-0
    attention, projections emitted two blocks deferred so attention
    matmuls lead the PE stream; PSUM packed exactly 8 banks/window

Self-contained: hardcodes all shapes; imports only numpy/ml_dtypes/concourse.
"""

import numpy as np
import ml_dtypes

import concourse.bass as bass  # noqa: F401
import concourse.bacc as bacc
import concourse.mybir as mybir
import concourse.tile as tile  # noqa: F401
from concourse.bass_utils import run_bass_kernel_spmd
from concourse.tile import TileContext

F16 = np.float16
BF16 = ml_dtypes.bfloat16

NUM_HEADS = 16
DIM = 2048
N = 4096
HD = 128          # head dim
P = 128           # partitions
NCORES = 8
HPC = 2           # heads per core
RMS_EPS = 1e-6
ROPE_THETA = 10000.0
SCALE = float(HD) ** -0.5

KO = DIM // P     # 16 contraction chunks
NB = N // 512     # 8 n-chunks of 512 (phase 1)
QB = N // 512     # 8 q-blocks of 512 (phase 2)
MI = N // P       # 32 m-chunks of 128
COLS = 3 * HPC    # 6 col chunks of 128 (q0,k0,v0,q1,k1,v1)
EXP_BIAS = -2.0   # softmax exp bias (cancels in numerator/denominator)

_CACHE = {}


# --------------------------------------------------------------------------
# host-side helpers
# --------------------------------------------------------------------------

def _rope_ext_tables(T, H, W):
    """cos_ext, sin_ext of shape (HD, N): extended interleaved RoPE tables.

    q_rot[d, n] = cos_ext[d, n] * q[d, n] + sin_ext[d, n] * q[pair(d), n]
    where pair(2i) = 2i+1, pair(2i+1) = 2i.
    """
    dt_ = HD // 2
    dh = HD // 4
    dw = HD - dt_ - dh

    def ax(L, d):
        inv = 1.0 / (ROPE_THETA ** (np.arange(0, d, 2, dtype=np.float32) / d))
        return np.arange(L, dtype=np.float32)[:, None] * inv

    ft = np.broadcast_to(ax(T, dt_)[:, None, None, :], (T, H, W, dt_ // 2))
    fh = np.broadcast_to(ax(H, dh)[None, :, None, :], (T, H, W, dh // 2))
    fw = np.broadcast_to(ax(W, dw)[None, None, :, :], (T, H, W, dw // 2))
    f = np.concatenate([ft, fh, fw], axis=-1).reshape(T * H * W, HD // 2)
    cos = np.cos(f).astype(np.float32).T   # (64, N)
    sin = np.sin(f).astype(np.float32).T   # (64, N)
    n_tot = T * H * W
    cos_ext = np.repeat(cos, 2, axis=0)    # (128, N)
    sin_ext = np.empty((HD, n_tot), np.float32)
    sin_ext[0::2] = -sin
    sin_ext[1::2] = sin
    return cos_ext, sin_ext


def _pair_swap(v):
    """Swap adjacent pairs of a (128,) vector."""
    return v.reshape(HD // 2, 2)[:, ::-1].reshape(HD)


# --------------------------------------------------------------------------
# device program
# --------------------------------------------------------------------------

def _build_nc(with_bias=False):
    nc = bacc.Bacc("TRN2", target_bir_lowering=False, debug=False)
    f32 = mybir.dt.float32
    f16 = mybir.dt.float16

    Exp = mybir.ActivationFunctionType.Exp
    Log = mybir.ActivationFunctionType.Ln
    Copy = mybir.ActivationFunctionType.Copy
    MULT = mybir.AluOpType.mult
    ADD = mybir.AluOpType.add

    xt = nc.dram_tensor("xt", [DIM, N], f16, kind="ExternalInput")
    wt = nc.dram_tensor("wt", [P, COLS, KO, P], f16, kind="ExternalInput")
    bqkv = nc.dram_tensor("bqkv", [P, COLS], f32, kind="ExternalInput")
    wp = nc.dram_tensor("wp", [HPC, P, DIM], f16, kind="ExternalInput")
    cosq = nc.dram_tensor("cosq", [P, N], f16, kind="ExternalInput")
    sinq = nc.dram_tensor("sinq", [P, N], f16, kind="ExternalInput")
    cosk = nc.dram_tensor("cosk", [P, N], f16, kind="ExternalInput")
    sink = nc.dram_tensor("sink", [P, N], f16, kind="ExternalInput")
    ones16 = nc.dram_tensor("ones16", [P, 1], f16, kind="ExternalInput")
    perm = nc.dram_tensor("perm", [P, P], f16, kind="ExternalInput")
    epsc = nc.dram_tensor("epsc", [1, 1], f32, kind="ExternalInput")
    ebias = nc.dram_tensor("ebias", [P, 1], f32, kind="ExternalInput")
    onesr = nc.dram_tensor("onesr", [1, P], f16, kind="ExternalInput")
    bqvr = nc.dram_tensor("bqvr", [1, COLS * P], f16, kind="ExternalInput")
    out_p = nc.dram_tensor("out_p", [N, DIM], f16, kind="ExternalOutput")

    xt_v = xt[:].rearrange("(ko p) n -> p ko n", p=P)    # (128, 16, 4096)

    from concourse.hw_specs import get_activation_tables
    tabs = get_activation_tables(nc.m.arch)
    need = {mybir.ActivationFunctionType.Exp, mybir.ActivationFunctionType.Ln,
            mybir.ActivationFunctionType.Copy}
    set_id = next((i for i, (nm, s) in enumerate(tabs.items())
                   if need.issubset(s)), None)

    with TileContext(nc) as tc:
      if set_id is not None:
          nc.scalar.add_instruction(mybir.InstLoadActFuncSet(
              name=nc.get_next_instruction_name(), act_func_set_id=set_id,
              ins=[], outs=[]))
      with tc.tile_pool(name="persist", bufs=1) as pers:
        wt_sb = pers.tile([P, COLS, KO, P], f16, tag="wt_sb")
        # column order: k0, v0, q0 first (phase-1 head-0 kv pass needs them)
        for c6 in (1, 2):
            nc.sync.dma_start(wt_sb[:, c6], wt[:, c6])
        wp_sb = [pers.tile([P, DIM], f16, tag=f"wp{h}", name=f"wp_sb{h}")
                 for h in range(HPC)]
        ones_sb = pers.tile([P, 1], f16, tag="ones_sb")
        nc.sync.dma_start(ones_sb[:], ones16[:])
        perm_sb = pers.tile([P, P], f16, tag="perm_sb")
        nc.sync.dma_start(perm_sb[:], perm[:])
        eps_sb = pers.tile([1, 1], f32, tag="eps_sb")
        nc.sync.dma_start(eps_sb[:], epsc[:])
        eb_sb = pers.tile([P, 1], f32, tag="eb_sb")
        nc.sync.dma_start(eb_sb[:], ebias[:])
        bq_sb = pers.tile([P, COLS], f32, tag="bq_sb")
        nc.sync.dma_start(bq_sb[:], bqkv[:])
        onesr_sb = pers.tile([1, P], f16, tag="onesr_sb")
        nc.sync.dma_start(onesr_sb[:], onesr[:])
        bqvr_sb = pers.tile([1, COLS * P], f16, tag="bqvr_sb")
        nc.sync.dma_start(bqvr_sb[:], bqvr[:])

        qT = [pers.tile([P, N], f16, tag=f"qT{h}", name=f"qT{h}")
              for h in range(HPC)]
        kT = [pers.tile([P, N], f16, tag=f"kT{h}", name=f"kT{h}")
              for h in range(HPC)]
        vn = [pers.tile([P, MI, HD], f16, tag=f"vn{h}", name=f"vn{h}")
              for h in range(HPC)]
        oT = [pers.tile([P, N], f16, tag=f"oT{h}", name=f"oT{h}")
              for h in range(HPC)]

        cs_dram = {("q", 0): cosq, ("q", 1): sinq,
                   ("k", 0): cosk, ("k", 1): sink}

        def phase1(hh, nb, p1, wk, ps_qkv, ps_qs, ps_sm, which="all"):
            """qkv + rmsnorm + rope (+ transposed v) for head hh, block nb.

            which: "all", "kv" (k column + v), or "q" (q column only).
            """
            nsl = slice(nb * 512, (nb + 1) * 512)
            xt_t = p1.tile([P, KO, 512], f16, tag="xt_t",
                           name=f"xt_{hh}_{nb}_{which}")
            nsp = 8 if (nb == 0 and which == "kv") else 2
            for kh in range(nsp):
                kw = KO // nsp
                nc.sync.dma_start(xt_t[:, kh * kw:(kh + 1) * kw, :],
                                  xt_v[:, kh * kw:(kh + 1) * kw, nsl])
            cs_t = {}
            tags = [] if which == "kv" else ["q"]
            if which != "q":
                tags.append("k")
            for tg in tags:
                for ci in range(2):
                    t = p1.tile([P, 512], f16, tag=f"cs_{tg}_{ci}",
                                name=f"cs_{tg}_{ci}")
                    nc.sync.dma_start(t[:], cs_dram[(tg, ci)][:, nsl])
                    cs_t[(tg, ci)] = t

            pqbs = {}
            for tag in tags:
                t3 = 0 if tag == "q" else 1
                col = hh * 3 + t3
                pq = ps_qkv.tile([P, 512], f32, tag="pq", name="pq")
                for k in range(KO):
                    nc.tensor.matmul(
                        pq[:],
                        lhsT=wt_sb[:, col, k, :],
                        rhs=xt_t[:, k],
                        start=(k == 0), stop=(k == KO - 1))
                # pqb: psum -> f16 sbuf (DVE); optional bias add
                pqb = wk.tile([P, 512], f16, tag="pqb", bufs=4)
                if with_bias:
                    nc.vector.tensor_scalar_add(pqb[:], pq[:],
                                                bq_sb[:, col:col + 1])
                else:
                    nc.vector.tensor_copy(pqb[:], pq[:])
                pqbs[tag] = pqb

            if which != "q":
                # v via transposed matmuls (fills PE while rms chains run)
                vcol = hh * 3 + 2
                pv = ps_qkv.tile([P, 4, HD], f32, tag="pq", name="pv")
                for c4 in range(4):
                    for k in range(KO):
                        nc.tensor.matmul(
                            pv[:, c4, :],
                            lhsT=xt_t[:, k, c4 * P:(c4 + 1) * P],
                            rhs=wt_sb[:, vcol, k, :],
                            start=(k == 0),
                            stop=(k == KO - 1 and not with_bias))
                    if with_bias:
                        nc.tensor.matmul(
                            pv[:, c4, :], lhsT=onesr_sb[0:1, :],
                            rhs=bqvr_sb[0:1, vcol * P:(vcol + 1) * P],
                            start=False, stop=True)
                dstv = vn[hh][:, nb * 4:(nb + 1) * 4, :]
                if hh == 0:
                    nc.scalar.activation(
                        dstv.rearrange("p a b -> p (a b)"),
                        pv[:].rearrange("p a b -> p (a b)"), Copy)
                else:
                    nc.vector.tensor_copy(
                        dstv.rearrange("p a b -> p (a b)"),
                        pv[:].rearrange("p a b -> p (a b)"))

            for tag in tags:
                t3 = 0 if tag == "q" else 1
                pqb = pqbs[tag]
                # rms: rs = (mean(pqb^2) + eps) ** -0.5 on (1, 512)
                q2b = wk.tile([P, 512], f16, tag="q2b", bufs=2)
                nc.gpsimd.tensor_tensor(q2b[:], pqb[:], pqb[:], MULT)
                pssq = ps_sm.tile([1, 512], f32, tag="qsm", name="pssq")
                nc.tensor.matmul(pssq[:], lhsT=ones_sb[:],
                                 rhs=q2b[:], start=True, stop=True)
                lms = wk.tile([1, 512], f32, tag="lms", bufs=2)
                nc.scalar.activation(lms[:], pssq[:], Log,
                                     bias=eps_sb[:], scale=1.0 / HD)
                rs = wk.tile([1, 512], f16, tag="rs", bufs=2)
                nc.scalar.activation(rs[:], lms[:], Exp, scale=-0.5)
                rsb = wk.tile([P, 512], f16, tag="rsb", bufs=3)
                nc.gpsimd.partition_broadcast(rsb[:], rs[:])
                # rope: dst = (pqb*cos + swap(pqb)*sin) * rsb
                pqs = ps_qs.tile([P, 512], f32, tag="qsm", name="pqs")
                nc.tensor.matmul(pqs[:], lhsT=perm_sb[:],
                                 rhs=pqb[:], start=True, stop=True)
                t1 = wk.tile([P, 512], f16, tag="t1", bufs=3)
                nc.gpsimd.tensor_tensor(t1[:], pqb[:], cs_t[(tag, 0)][:],
                                        MULT)
                t2 = wk.tile([P, 512], f16, tag="t2", bufs=3)
                nc.vector.tensor_tensor(t2[:], pqs[:], cs_t[(tag, 1)][:],
                                        MULT)
                t3t = wk.tile([P, 512], f16, tag="t3", bufs=2)
                nc.vector.tensor_tensor(t3t[:], t1[:], t2[:], ADD)
                dst = qT[hh] if t3 == 0 else kT[hh]
                nc.vector.tensor_tensor(dst[:, nsl], t3t[:], rsb[:], MULT)

            return

        def attention(h, qb, p2, p2s, ps_s, ps_o, ps_den,
                      pe_den=False):
            qsl = slice(qb * 512, (qb + 1) * 512)
            po = ps_o.tile([P, 512], f32, tag="po")
            pden_pe = (ps_den.tile([1, 512], f32, tag="qsm", name="pden_pe")
                       if pe_den else None)
            accs = [None, None, None, None]
            for mp in range(MI // 2):
                m0 = 2 * mp
                ps = ps_s.tile([P, 2, 512], f32, tag="ps")
                for u in range(2):
                    nc.tensor.matmul(
                        ps[:, u, :],
                        lhsT=kT[h][:, (m0 + u) * P:(m0 + u + 1) * P],
                        rhs=qT[h][:, qsl], start=True, stop=True)
                at = p2.tile([P, 2, 512], f16, tag="at", bufs=8)
                nc.scalar.activation(
                    at[:].rearrange("p a b -> p (a b)"),
                    ps[:].rearrange("p a b -> p (a b)"),
                    Exp, scale=SCALE, bias=eb_sb[:])
                for u in range(2):
                    nc.tensor.matmul(
                        po[:], lhsT=vn[h][:, m0 + u, :], rhs=at[:, u, :],
                        start=(m0 + u == 0), stop=(m0 + u == MI - 1))
                # denominator tree on DVE: pair-sum then 4 rotating accs
                tadd = p2.tile([P, 512], f16, tag="tadd", bufs=6)
                nc.vector.tensor_tensor(tadd[:], at[:, 0, :], at[:, 1, :],
                                        ADD)
                g = mp % 4
                if accs[g] is None:
                    accs[g] = tadd
                else:
                    nacc = p2.tile([P, 512], f16, tag=f"acc{g}", bufs=2)
                    nc.vector.tensor_tensor(nacc[:], accs[g][:], tadd[:],
                                            ADD)
                    accs[g] = nacc
            if pe_den:
                pden = pden_pe
            else:
                s01 = p2.tile([P, 512], f16, tag="s01", bufs=2)
                nc.vector.tensor_tensor(s01[:], accs[0][:], accs[1][:], ADD)
                s23 = p2.tile([P, 512], f16, tag="s23", bufs=2)
                nc.vector.tensor_tensor(s23[:], accs[2][:], accs[3][:], ADD)
                sall = p2.tile([P, 512], f16, tag="sall", bufs=2)
                nc.vector.tensor_tensor(sall[:], s01[:], s23[:], ADD)
                pden = ps_den.tile([1, 512], f32, tag="qsm", name="pden")
                _p = tc.cur_priority
                tc.cur_priority = _p + 250
                nc.tensor.matmul(pden[:], lhsT=ones_sb[:], rhs=sall[:],
                                 start=True, stop=True)
                tc.cur_priority = _p
            po_sb = p2s.tile([P, 512], f16, tag="po_sb", bufs=2)
            with tc.high_priority(offset=60):
                if h == 0:
                    nc.scalar.activation(po_sb[:], po[:], Copy)
                else:
                    nc.vector.tensor_copy(po_sb[:], po[:])
            rden = p2s.tile([1, 512], f32, tag="rden")
            nc.vector.reciprocal_approx_fast(rden[:], pden[:])
            rdb = p2s.tile([P, 512], f32, tag="rdb")
            nc.gpsimd.partition_broadcast(rdb[:], rden[:])
            nc.vector.tensor_tensor(oT[h][:, qsl], po_sb[:], rdb[:], MULT)

        def proj(qb, p3, ps_out):
            for qc in range(qb * 4, qb * 4 + 4):
                osb = p3.tile([P, DIM], f16, tag="osb")
                for ob in range(4):
                    obsl = slice(ob * 512, (ob + 1) * 512)
                    pout = ps_out.tile([P, 512], f32, tag="pout", bufs=2)
                    for h in range(HPC):
                        nc.tensor.matmul(
                            pout[:],
                            lhsT=oT[h][:, qc * P:(qc + 1) * P],
                            rhs=wp_sb[h][:, obsl],
                            start=(h == 0), stop=(h == HPC - 1))
                    if ob % 2 == 0:
                        nc.scalar.activation(osb[:, obsl], pout[:], Copy)
                    else:
                        nc.vector.tensor_copy(osb[:, obsl], pout[:])
                nc.sync.dma_start(out_p[qc * P:(qc + 1) * P, :], osb[:])

        for h in range(HPC):
            nc.sync.dma_start(wp_sb[h][:], wp[h])
        with tc.tile_pool(name="p2_sb", bufs=2) as p2, \
             tc.tile_pool(name="p2_sm", bufs=2) as p2s, \
             tc.tile_pool(name="ps_s", bufs=2, space="PSUM") as ps_s, \
             tc.tile_pool(name="ps_o", bufs=1, space="PSUM") as ps_o, \
             tc.tile_pool(name="ps_den", bufs=1, space="PSUM") as ps_den:
            ps_qs = ps_den
            ps_sm = ps_den
            with tc.tile_pool(name="p1_sb", bufs=2) as p1, \
                 tc.tile_pool(name="p1_wk", bufs=2) as wk, \
                 tc.tile_pool(name="ps_qkv", bufs=2, space="PSUM") as ps_qkv:
                phase1(0, 0, p1, wk, ps_qkv, ps_qs, ps_sm, which="kv")
                for c6 in (0, 4, 5, 3):
                    nc.sync.dma_start(wt_sb[:, c6], wt[:, c6])
                for nb in range(1, NB):
                    phase1(0, nb, p1, wk, ps_qkv, ps_qs, ps_sm, which="kv")
                phase1(0, 0, p1, wk, ps_qkv, ps_qs, ps_sm, which="q")
                for qb in range(QB):
                    if qb + 1 < QB:
                        phase1(0, qb + 1, p1, wk, ps_qkv, ps_qs, ps_sm,
                               which="q")
                    phase1(1, qb, p1, wk, ps_qkv, ps_qs, ps_sm)
                    attention(0, qb, p2, p2s, ps_s, ps_o, ps_den)
            with tc.tile_pool(name="p3_sb", bufs=3) as p3, \
                 tc.tile_pool(name="ps_out", bufs=1, space="PSUM") as ps_out:
                for qb in range(QB):
                    attention(1, qb, p2, p2s, ps_s, ps_o, ps_den)
                    if qb > 2:
                        proj(qb - 3, p3, ps_out)
                for qb in range(QB - 3, QB):
                    proj(qb, p3, ps_out)

    nc.compile()
    return nc


# --------------------------------------------------------------------------
# host wrapper
# --------------------------------------------------------------------------

def _prep_in_maps(x, qkv_w, qkv_b, q_norm_w, k_norm_w, proj_w, T, H, W):
    x2 = np.ascontiguousarray(x[0].T).astype(F16)           # (2048, 4096)
    cos_ext, sin_ext = _rope_ext_tables(T, H, W)
    wq = q_norm_w.astype(np.float32)
    wk_ = k_norm_w.astype(np.float32)
    cosq = (cos_ext * wq[:, None]).astype(F16)
    sinq = (sin_ext * _pair_swap(wq)[:, None]).astype(F16)
    cosk = (cos_ext * wk_[:, None]).astype(F16)
    sink = (sin_ext * _pair_swap(wk_)[:, None]).astype(F16)

    ones_16 = np.ones((P, 1), F16)
    permm = np.zeros((P, P), np.float32)
    idx = np.arange(P)
    pair = idx ^ 1
    permm[pair, idx] = 1.0  # psum_qs[m,n] = sum_k perm[k,m] q[k,n] = q[pair(m),n]
    permm = permm.astype(F16)

    in_maps = []
    for c in range(NCORES):
        h0 = HPC * c
        blocks = []
        bias_blocks = []
        for h in (h0, h0 + 1):
            for t3 in range(3):
                r0 = t3 * DIM + h * HD
                blocks.append(qkv_w[r0:r0 + HD])
                bias_blocks.append(qkv_b[r0:r0 + HD])
        wt_c = np.stack(blocks, axis=0)                     # (6, 128, 2048)
        # (P, COLS, KO, P): [p, c6, ko, j] = W[c6][j, ko*128+p]
        wt_c = np.ascontiguousarray(
            wt_c.reshape(COLS, P, KO, P).transpose(3, 0, 2, 1)).astype(F16)
        bq_c = np.stack(bias_blocks, axis=1).astype(np.float32)  # (128, 6)
        wp_c = np.stack(
            [np.ascontiguousarray(proj_w[:, (h0 + h) * HD:(h0 + h + 1) * HD].T)
             for h in range(HPC)], axis=0).astype(F16)      # (2, 128, 2048)
        in_maps.append({
            "xt": x2, "wt": wt_c, "bqkv": bq_c, "wp": wp_c,
            "cosq": cosq, "sinq": sinq, "cosk": cosk, "sink": sink,
            "ones16": ones_16, "perm": permm,
            "epsc": np.full((1, 1), RMS_EPS, np.float32),
            "onesr": np.ones((1, P), F16),
            "bqvr": np.concatenate(bias_blocks)[None, :].astype(F16),
            "ebias": np.full((P, 1), EXP_BIAS, np.float32),
        })
    return in_maps


def kernel(x, qkv_w, qkv_b, q_norm_w, k_norm_w, proj_w, proj_b, T, H, W):
    x = np.asarray(x)
    T, H, W = int(T), int(H), int(W)
    assert x.shape == (1, N, DIM) and T * H * W == N

    qkv_b = np.asarray(qkv_b)
    with_bias = bool(np.any(qkv_b))
    key = ("nc", with_bias)
    if key not in _CACHE:
        _CACHE[key] = _build_nc(with_bias=with_bias)
        _CACHE["nc"] = _CACHE[key]
    nc = _CACHE[key]

    in_maps = _prep_in_maps(
        x, np.asarray(qkv_w), qkv_b, np.asarray(q_norm_w),
        np.asarray(k_norm_w), np.asarray(proj_w), T, H, W)

    res = run_bass_kernel_spmd(nc, in_maps, core_ids=list(range(NCORES)))
    out = np.zeros((N, DIM), np.float64)
    for c in range(NCORES):
        out += res.results[c]["out_p"].astype(np.float64)
    out = out.astype(np.float32) + np.asarray(proj_b, np.float32)[None, :]
    return out[None].astype(x.dtype)
